# revision 21
# baseline (speedup 1.0000x reference)
"""Trainium2 Bass kernel for nn_CustomSelectAttention (topk_masking).

Computes, for each batch row b:
  read_n  = q[b,n,:] @ w_read[n]   (n = 0,1 only — slots 2-4 unused)
  write_m = k[b,m,:] @ w_write[m]  (m = 0..4)
  s_n[m]  = read_n . write_m / 4 + gumbel_noise_n[b,m]
  out_n   = one_hot(argmax_m s_n[m])
Returns (g1, g2), each [32768, 5] float32 — equal in value to the
reference's straight-through hard gumbel-softmax outputs.

Strategy: pure data parallel over 8 NeuronCores (4096 rows each).
Per core, per 512-row supertile: PE transposes q/k 128x128 chunks (f32,
bit-exact) into PSUM staging, ACT/DVE copies round them to float32r in
SBUF, then weights-stationary float32r matmuls (full PE rate, N=512)
accumulate all 7 projections into one [128, 512] PSUM tile — packed as
4 col-groups of M=32 via block-diagonal weight pairs so output base
partitions stay 32-aligned. A PE transpose-back restores batch-major
layout and the DVE computes scores, adds host-precomputed Gumbel noise
(jax-bit-exact), and emits the one-hot via is_equal against the row max.
"""
import os
import numpy as np

import concourse.bass as bass  # noqa: F401  (engine namespaces live on nc)
import concourse.mybir as mybir
import concourse.tile as tile
from concourse import bacc
from concourse.bass_utils import run_bass_kernel_spmd
from concourse.masks import make_identity

F32 = mybir.dt.float32
F32R = mybir.dt.float32r
P = 128
NCORES = 8
B = 32768
BC = B // NCORES          # rows per core = 4096
NBS = BC // P             # b-subtiles per core = 32
NST = NBS // 4            # supertiles per core = 8 (512 rows each)
D = 512
DK = 16
NW = 5


def gumbel_noise(seed, shape):
    """Bit-exact match of the reference's gumbel noise: computed with the
    same jax ops on the same platform."""
    import jax
    import jax.numpy as jnp
    u = jax.random.uniform(jax.random.key(seed), shape, minval=1e-10, maxval=1.0)
    g = -jnp.log(-jnp.log(u))
    return np.asarray(g, dtype=np.float32)


# Projection t -> (group g, member) packing: pairs (0,1) (2,3) (4,5) share a
# 32-wide col-group via block-diagonal weights; t=6 rides alone in group 3.
def _gm(t):
    return (t // 2, t % 2) if t < 6 else (3, 0)


def _slot(g, cc):
    return g * 8 + cc if g < 3 else 24 + cc


def _build():
    nc = bacc.Bacc(trn_type="TRN2", target_bir_lowering=False, debug=False)
    q_in = nc.dram_tensor("q", [BC, 1024], F32R, kind="ExternalInput").ap()
    k_in = nc.dram_tensor("k", [BC, 2560], F32R, kind="ExternalInput").ap()
    w_in = nc.dram_tensor("w", [P, 896], F32, kind="ExternalInput").ap()
    nz_in = nc.dram_tensor("noise", [P, 2 * NW * NBS], F32, kind="ExternalInput").ap()
    o1 = nc.dram_tensor("out1", [P, NW * NBS], F32, kind="ExternalOutput").ap()
    o2 = nc.dram_tensor("out2", [P, NW * NBS], F32, kind="ExternalOutput").ap()

    qv = q_in.rearrange("(n p) d -> n p d", p=P)   # [32, 128, 1024]
    kv = k_in.rearrange("(n p) d -> n p d", p=P)   # [32, 128, 2560]

    with tile.TileContext(nc, pool_alloc_mode="queue") as tc:
        with (
            tc.tile_pool(name="const", bufs=1) as constp,
            tc.tile_pool(name="qin", bufs=10) as qinp,
            tc.tile_pool(name="kin", bufs=10) as kinp,
            tc.tile_pool(name="stage", bufs=5, space="PSUM") as stagep,
            tc.tile_pool(name="qt", bufs=12) as qtp,
            tc.tile_pool(name="pT", bufs=2, space="PSUM") as pTp,
            tc.tile_pool(name="pTs", bufs=2) as pTsp,
            tc.tile_pool(name="pB", bufs=1, space="PSUM") as pBp,
            tc.tile_pool(name="pBs", bufs=2) as pBsp,
            tc.tile_pool(name="ep", bufs=4) as epp,
        ):
            ident_f = constp.tile([P, P], F32)
            make_identity(nc, ident_f)
            ident = constp.tile([P, P], F32R)
            nc.vector.tensor_copy(ident[:], ident_f[:])
            w_sb = constp.tile([P, 896], F32)
            nc.sync.dma_start(w_sb[:], w_in[:])
            w_r = constp.tile([P, 896], F32R)
            nc.vector.tensor_copy(w_r[:], w_sb[:])          # rounds f32 -> f32r
            nz_sb = constp.tile([P, 2 * NW * NBS], F32)
            nc.sync.dma_start(nz_sb[:], nz_in[:])
            out1_sb = constp.tile([P, NW * NBS], F32)
            out2_sb = constp.tile([P, NW * NBS], F32)

            copy_toggle = 0
            for st in range(NST):
                qin = []
                kin = []
                for s in range(4):
                    j = 4 * st + s
                    qt_t = qinp.tile([P, 1024], F32R, tag="qin", name=f"qin_{j}")
                    for sl in range(2):
                        nc.sync.dma_start(qt_t[:, 512 * sl:512 * (sl + 1)],
                                          qv[j][:, 512 * sl:512 * (sl + 1)])
                    qin.append(qt_t)
                for s in range(4):
                    j = 4 * st + s
                    kt_t = kinp.tile([P, 2560], F32R, tag="kin", name=f"kin_{j}")
                    nc.sync.dma_start(kt_t[:, 0:1024], kv[j][:, 0:1024])
                    nc.sync.dma_start(kt_t[:, 1024:2560], kv[j][:, 1024:2560])
                    kin.append(kt_t)

                projT = [None] * 4
                pending = []

                def flush_mm():
                    g_, slot_, cc_, qt_ = pending.pop(0)
                    last_ = (cc_ == 7) if g_ < 3 else (cc_ == 3)
                    nc.tensor.matmul(projT[g_][:],
                                     w_r[:, 32 * slot_: 32 * (slot_ + 1)],
                                     qt_,
                                     start=(cc_ == 0), stop=last_)

                for t in range(7):
                    g, member = _gm(t)
                    for c in range(4):
                        cc = member * 4 + c
                        slot = _slot(g, cc)
                        if projT[g] is None:
                            projT[g] = pTp.tile([32, D], F32, tag="pT", name=f"pT{g}_{st}")
                        stg = stagep.tile([P, D], F32R, tag="stage", name=f"stg_{st}_{t}_{c}")
                        for s in range(4):
                            if t < 2:
                                src = qin[s][:, 512 * t + 128 * c: 512 * t + 128 * (c + 1)]
                            else:
                                src = kin[s][:, 512 * (t - 2) + 128 * c: 512 * (t - 2) + 128 * (c + 1)]
                            nc.tensor.transpose(stg[:, 128 * s:128 * (s + 1)], src, ident[:])
                        qt_r = qtp.tile([P, D], F32R, tag="qt", name=f"qt_{st}_{t}_{c}")
                        if copy_toggle % 2 == 0:
                            nc.vector.tensor_copy(qt_r[:], stg[:])
                        else:
                            nc.scalar.copy(qt_r[:], stg[:])
                        copy_toggle += 1
                        pending.append((g, slot, cc, qt_r[:]))
                        if len(pending) > 2:
                            flush_mm()
                while pending:
                    flush_mm()

                pTs_all = pTsp.tile([P, D], F32, tag="pTs", name=f"pTs_{st}")
                for g in range(4):
                    nc.scalar.copy(pTs_all[32 * g:32 * (g + 1), :], projT[g][:])

                for s in range(4):
                    j = 4 * st + s
                    pB = pBp.tile([P, P], F32, tag="pB", name=f"pB_{j}")
                    nc.tensor.transpose(pB[:], pTs_all[:, 128 * s:128 * (s + 1)], ident_f[:])
                    pBs = pBsp.tile([P, P], F32, tag="pBs", name=f"pBs_{j}")
                    nc.vector.tensor_copy(pBs[:], pB[:])

                    rr = pBs[:, 0:32].rearrange("p (s k) -> p s k", s=2).unsqueeze(2).broadcast_to([P, 2, NW, DK])
                    wv4 = pBs[:, 32:112].rearrange("p (m k) -> p m k", m=NW).unsqueeze(1).broadcast_to([P, 2, NW, DK])
                    prod = epp.tile([P, 2, NW, DK], F32, tag="prod", name=f"prod_{j}")
                    nc.gpsimd.tensor_tensor(prod[:], rr, wv4, op=mybir.AluOpType.mult)
                    sc = epp.tile([P, 2 * NW], F32, tag="sc", name=f"sc_{j}")
                    nc.vector.reduce_sum(sc[:].rearrange("p (s m) -> p s m", s=2), prod[:],
                                         axis=mybir.AxisListType.X)
                    scn = epp.tile([P, 2 * NW], F32, tag="scn", name=f"scn_{j}")
                    nc.gpsimd.tensor_tensor(scn[:], sc[:], nz_sb[:, 2 * NW * j: 2 * NW * (j + 1)],
                                            op=mybir.AluOpType.add)
                    mx = epp.tile([P, 2], F32, tag="mx", name=f"mx_{j}")
                    nc.vector.reduce_max(mx[:], scn[:].rearrange("p (s m) -> p s m", s=2),
                                         axis=mybir.AxisListType.X)
                    nc.vector.tensor_scalar(out1_sb[:, NW * j: NW * (j + 1)], scn[:, 0:NW],
                                            scalar1=mx[:, 0:1], scalar2=None,
                                            op0=mybir.AluOpType.is_equal)
                    nc.vector.tensor_scalar(out2_sb[:, NW * j: NW * (j + 1)], scn[:, NW:2 * NW],
                                            scalar1=mx[:, 1:2], scalar2=None,
                                            op0=mybir.AluOpType.is_equal)

            nc.sync.dma_start(o1[:], out1_sb[:])
            nc.sync.dma_start(o2[:], out2_sb[:])

    nc.compile()
    return nc


_NC_CACHE = []


def _pack_weights(w_read, w_write):
    projw = [np.asarray(w_read[0]) * np.float32(0.25),
             np.asarray(w_read[1]) * np.float32(0.25)] + \
            [np.asarray(w_write[m]) for m in range(NW)]
    W = np.zeros((28, P, 32), np.float32)
    for t in range(7):
        g, member = _gm(t)
        for c in range(4):
            cc = member * 4 + c
            slot = _slot(g, cc)
            W[slot, :, 16 * member:16 * (member + 1)] = projw[t][128 * c:128 * (c + 1), :]
    return np.ascontiguousarray(W.transpose(1, 0, 2).reshape(P, 896))


def kernel(q, k, w_read, w_write):
    q = np.asarray(q, dtype=np.float32)
    k = np.asarray(k, dtype=np.float32)
    w_read = np.asarray(w_read, dtype=np.float32)
    w_write = np.asarray(w_write, dtype=np.float32)

    wt = _pack_weights(w_read, w_write)
    g1 = gumbel_noise(42, (B, NW))
    g2 = gumbel_noise(43, (B, NW))

    in_maps = []
    for ci in range(NCORES):
        rows = slice(BC * ci, BC * (ci + 1))
        qs = np.ascontiguousarray(q[rows, 0:2, :].reshape(BC, 1024))
        ks = np.ascontiguousarray(k[rows].reshape(BC, 2560))
        n1 = g1[rows].reshape(NBS, P, NW).transpose(1, 0, 2)   # [128, 32, 5]
        n2 = g2[rows].reshape(NBS, P, NW).transpose(1, 0, 2)
        nz = np.ascontiguousarray(np.stack([n1, n2], axis=2).reshape(P, 2 * NW * NBS))
        in_maps.append({"q": qs, "k": ks, "w": wt, "noise": nz})

    if not _NC_CACHE:
        _NC_CACHE.append(_build())
    nc = _NC_CACHE[0]

    trace = bool(int(os.environ.get("BASS_KERNEL_TRACE", "0")))
    res = run_bass_kernel_spmd(nc, in_maps, core_ids=list(range(NCORES)), trace=trace)
    kernel.last_exec_time_ns = res.exec_time_ns

    g1_out = np.empty((B, NW), np.float32)
    g2_out = np.empty((B, NW), np.float32)
    for ci in range(NCORES):
        rows = slice(BC * ci, BC * (ci + 1))
        r = res.results[ci]
        g1_out[rows] = r["out1"].reshape(P, NBS, NW).transpose(1, 0, 2).reshape(BC, NW)
        g2_out[rows] = r["out2"].reshape(P, NBS, NW).transpose(1, 0, 2).reshape(BC, NW)
    return (g1_out, g2_out)


kernel.last_exec_time_ns = None


# revision 22
# speedup vs baseline: 1.1828x; 1.1828x over previous
"""Trainium2 Bass kernel for nn_CustomSelectAttention (topk_masking).

Computes, for each batch row b:
  read_n  = q[b,n,:] @ w_read[n]   (n = 0,1 only — slots 2-4 unused)
  write_m = k[b,m,:] @ w_write[m]  (m = 0..4)
  s_n[m]  = read_n . write_m / 4 + gumbel_noise_n[b,m]
  out_n   = one_hot(argmax_m s_n[m])
Returns (g1, g2), each [32768, 5] float32 — equal in value to the
reference's straight-through hard gumbel-softmax outputs.

Strategy: pure data parallel over 8 NeuronCores (4096 rows each).
Per core, per 512-row supertile: PE transposes q/k 128x128 chunks (f32,
bit-exact) into PSUM staging, ACT/DVE copies round them to float32r in
SBUF, then weights-stationary float32r matmuls (full PE rate, N=512)
accumulate all 7 projections into one [128, 512] PSUM tile — packed as
4 col-groups of M=32 via block-diagonal weight pairs so output base
partitions stay 32-aligned. A PE transpose-back restores batch-major
layout and the DVE computes scores, adds host-precomputed Gumbel noise
(jax-bit-exact), and emits the one-hot via is_equal against the row max.
"""
import os
import numpy as np

import concourse.bass as bass  # noqa: F401  (engine namespaces live on nc)
import concourse.mybir as mybir
import concourse.tile as tile
from concourse import bacc
from concourse.bass_utils import run_bass_kernel_spmd
from concourse.masks import make_identity

F32 = mybir.dt.float32
F32R = mybir.dt.float32r
P = 128
NCORES = 8
B = 32768
BC = B // NCORES          # rows per core = 4096
NBS = BC // P             # b-subtiles per core = 32
NST = NBS // 4            # supertiles per core = 8 (512 rows each)
D = 512
DK = 16
NW = 5


def gumbel_noise(seed, shape):
    """Bit-exact match of the reference's gumbel noise: computed with the
    same jax ops on the same platform."""
    import jax
    import jax.numpy as jnp
    u = jax.random.uniform(jax.random.key(seed), shape, minval=1e-10, maxval=1.0)
    g = -jnp.log(-jnp.log(u))
    return np.asarray(g, dtype=np.float32)


# Projection t -> (group g, member) packing: pairs (0,1) (2,3) (4,5) share a
# 32-wide col-group via block-diagonal weights; t=6 rides alone in group 3.
def _gm(t):
    return (t // 2, t % 2) if t < 6 else (3, 0)


def _slot(g, cc):
    return g * 8 + cc if g < 3 else 24 + cc


def _build():
    nc = bacc.Bacc(trn_type="TRN2", target_bir_lowering=False, debug=False)
    q_in = nc.dram_tensor("q", [BC, 1024], F32R, kind="ExternalInput").ap()
    k_in = nc.dram_tensor("k", [BC, 2560], F32R, kind="ExternalInput").ap()
    w_in = nc.dram_tensor("w", [P, 896], F32, kind="ExternalInput").ap()
    nz_in = nc.dram_tensor("noise", [P, 2 * NW * NBS], F32, kind="ExternalInput").ap()
    o1 = nc.dram_tensor("out1", [P, NW * NBS], F32, kind="ExternalOutput").ap()
    o2 = nc.dram_tensor("out2", [P, NW * NBS], F32, kind="ExternalOutput").ap()

    qv = q_in.rearrange("(n p) d -> n p d", p=P)   # [32, 128, 1024]
    kv = k_in.rearrange("(n p) d -> n p d", p=P)   # [32, 128, 2560]

    with tile.TileContext(nc) as tc:
        with (
            tc.tile_pool(name="const", bufs=1) as constp,
            tc.tile_pool(name="qin", bufs=8) as qinp,
            tc.tile_pool(name="kin", bufs=11) as kinp,
            tc.tile_pool(name="stage", bufs=5, space="PSUM") as stagep,
            tc.tile_pool(name="qt", bufs=12) as qtp,
            tc.tile_pool(name="pT", bufs=2, space="PSUM") as pTp,
            tc.tile_pool(name="pTs", bufs=2) as pTsp,
            tc.tile_pool(name="pB", bufs=1, space="PSUM") as pBp,
            tc.tile_pool(name="pBs", bufs=2) as pBsp,
            tc.tile_pool(name="ep", bufs=4) as epp,
        ):
            ident_f = constp.tile([P, P], F32)
            make_identity(nc, ident_f)
            ident = constp.tile([P, P], F32R)
            nc.vector.tensor_copy(ident[:], ident_f[:])
            w_sb = constp.tile([P, 896], F32)
            nc.sync.dma_start(w_sb[:], w_in[:])
            w_r = constp.tile([P, 896], F32R)
            nc.vector.tensor_copy(w_r[:], w_sb[:])          # rounds f32 -> f32r
            nz_sb = constp.tile([P, 2 * NW * NBS], F32)
            nc.sync.dma_start(nz_sb[:], nz_in[:])
            out1_sb = constp.tile([P, NW * NBS], F32)
            out2_sb = constp.tile([P, NW * NBS], F32)

            copy_toggle = 0
            for st in range(NST):
                qin = []
                kin = []
                for s in range(4):
                    j = 4 * st + s
                    qt_t = qinp.tile([P, 1024], F32R, tag="qin", name=f"qin_{j}")
                    for sl in range(2):
                        nc.sync.dma_start(qt_t[:, 512 * sl:512 * (sl + 1)],
                                          qv[j][:, 512 * sl:512 * (sl + 1)])
                    qin.append(qt_t)
                for s in range(4):
                    j = 4 * st + s
                    kt_t = kinp.tile([P, 2560], F32R, tag="kin", name=f"kin_{j}")
                    nc.sync.dma_start(kt_t[:, 0:1024], kv[j][:, 0:1024])
                    nc.sync.dma_start(kt_t[:, 1024:2560], kv[j][:, 1024:2560])
                    kin.append(kt_t)

                projT = [None] * 4
                pending = []

                def flush_mm():
                    g_, slot_, cc_, qt_ = pending.pop(0)
                    last_ = (cc_ == 7) if g_ < 3 else (cc_ == 3)
                    nc.tensor.matmul(projT[g_][:],
                                     w_r[:, 32 * slot_: 32 * (slot_ + 1)],
                                     qt_,
                                     start=(cc_ == 0), stop=last_)

                for t in range(7):
                    g, member = _gm(t)
                    for c in range(4):
                        cc = member * 4 + c
                        slot = _slot(g, cc)
                        if projT[g] is None:
                            projT[g] = pTp.tile([32, D], F32, tag="pT", name=f"pT{g}_{st}")
                        stg = stagep.tile([P, D], F32R, tag="stage", name=f"stg_{st}_{t}_{c}")
                        for s in range(4):
                            if t < 2:
                                src = qin[s][:, 512 * t + 128 * c: 512 * t + 128 * (c + 1)]
                            else:
                                src = kin[s][:, 512 * (t - 2) + 128 * c: 512 * (t - 2) + 128 * (c + 1)]
                            nc.tensor.transpose(stg[:, 128 * s:128 * (s + 1)], src, ident[:])
                        qt_r = qtp.tile([P, D], F32R, tag="qt", name=f"qt_{st}_{t}_{c}")
                        if copy_toggle % 2 == 0:
                            nc.vector.tensor_copy(qt_r[:], stg[:])
                        else:
                            nc.scalar.copy(qt_r[:], stg[:])
                        copy_toggle += 1
                        pending.append((g, slot, cc, qt_r[:]))
                        if len(pending) > 2:
                            flush_mm()
                while pending:
                    flush_mm()

                pTs_all = pTsp.tile([P, D], F32, tag="pTs", name=f"pTs_{st}")
                for g in range(4):
                    nc.scalar.copy(pTs_all[32 * g:32 * (g + 1), :], projT[g][:])

                for s in range(4):
                    j = 4 * st + s
                    pB = pBp.tile([P, P], F32, tag="pB", name=f"pB_{j}")
                    nc.tensor.transpose(pB[:], pTs_all[:, 128 * s:128 * (s + 1)], ident_f[:])
                    pBs = pBsp.tile([P, P], F32, tag="pBs", name=f"pBs_{j}")
                    nc.vector.tensor_copy(pBs[:], pB[:])

                    rr = pBs[:, 0:32].rearrange("p (s k) -> p s k", s=2).unsqueeze(2).broadcast_to([P, 2, NW, DK])
                    wv4 = pBs[:, 32:112].rearrange("p (m k) -> p m k", m=NW).unsqueeze(1).broadcast_to([P, 2, NW, DK])
                    prod = epp.tile([P, 2, NW, DK], F32, tag="prod", name=f"prod_{j}")
                    nc.gpsimd.tensor_tensor(prod[:], rr, wv4, op=mybir.AluOpType.mult)
                    sc = epp.tile([P, 2 * NW], F32, tag="sc", name=f"sc_{j}")
                    nc.vector.reduce_sum(sc[:].rearrange("p (s m) -> p s m", s=2), prod[:],
                                         axis=mybir.AxisListType.X)
                    scn = epp.tile([P, 2 * NW], F32, tag="scn", name=f"scn_{j}")
                    nc.gpsimd.tensor_tensor(scn[:], sc[:], nz_sb[:, 2 * NW * j: 2 * NW * (j + 1)],
                                            op=mybir.AluOpType.add)
                    mx = epp.tile([P, 2], F32, tag="mx", name=f"mx_{j}")
                    nc.vector.reduce_max(mx[:], scn[:].rearrange("p (s m) -> p s m", s=2),
                                         axis=mybir.AxisListType.X)
                    nc.vector.tensor_scalar(out1_sb[:, NW * j: NW * (j + 1)], scn[:, 0:NW],
                                            scalar1=mx[:, 0:1], scalar2=None,
                                            op0=mybir.AluOpType.is_equal)
                    nc.vector.tensor_scalar(out2_sb[:, NW * j: NW * (j + 1)], scn[:, NW:2 * NW],
                                            scalar1=mx[:, 1:2], scalar2=None,
                                            op0=mybir.AluOpType.is_equal)

            nc.sync.dma_start(o1[:], out1_sb[:])
            nc.sync.dma_start(o2[:], out2_sb[:])

    nc.compile()
    return nc


_NC_CACHE = []


def _pack_weights(w_read, w_write):
    projw = [np.asarray(w_read[0]) * np.float32(0.25),
             np.asarray(w_read[1]) * np.float32(0.25)] + \
            [np.asarray(w_write[m]) for m in range(NW)]
    W = np.zeros((28, P, 32), np.float32)
    for t in range(7):
        g, member = _gm(t)
        for c in range(4):
            cc = member * 4 + c
            slot = _slot(g, cc)
            W[slot, :, 16 * member:16 * (member + 1)] = projw[t][128 * c:128 * (c + 1), :]
    return np.ascontiguousarray(W.transpose(1, 0, 2).reshape(P, 896))


def kernel(q, k, w_read, w_write):
    q = np.asarray(q, dtype=np.float32)
    k = np.asarray(k, dtype=np.float32)
    w_read = np.asarray(w_read, dtype=np.float32)
    w_write = np.asarray(w_write, dtype=np.float32)

    wt = _pack_weights(w_read, w_write)
    g1 = gumbel_noise(42, (B, NW))
    g2 = gumbel_noise(43, (B, NW))

    in_maps = []
    for ci in range(NCORES):
        rows = slice(BC * ci, BC * (ci + 1))
        qs = np.ascontiguousarray(q[rows, 0:2, :].reshape(BC, 1024))
        ks = np.ascontiguousarray(k[rows].reshape(BC, 2560))
        n1 = g1[rows].reshape(NBS, P, NW).transpose(1, 0, 2)   # [128, 32, 5]
        n2 = g2[rows].reshape(NBS, P, NW).transpose(1, 0, 2)
        nz = np.ascontiguousarray(np.stack([n1, n2], axis=2).reshape(P, 2 * NW * NBS))
        in_maps.append({"q": qs, "k": ks, "w": wt, "noise": nz})

    if not _NC_CACHE:
        _NC_CACHE.append(_build())
    nc = _NC_CACHE[0]

    trace = bool(int(os.environ.get("BASS_KERNEL_TRACE", "0")))
    res = run_bass_kernel_spmd(nc, in_maps, core_ids=list(range(NCORES)), trace=trace)
    kernel.last_exec_time_ns = res.exec_time_ns

    g1_out = np.empty((B, NW), np.float32)
    g2_out = np.empty((B, NW), np.float32)
    for ci in range(NCORES):
        rows = slice(BC * ci, BC * (ci + 1))
        r = res.results[ci]
        g1_out[rows] = r["out1"].reshape(P, NBS, NW).transpose(1, 0, 2).reshape(BC, NW)
        g2_out[rows] = r["out2"].reshape(P, NBS, NW).transpose(1, 0, 2).reshape(BC, NW)
    return (g1_out, g2_out)


kernel.last_exec_time_ns = None


# revision 23
# speedup vs baseline: 1.2206x; 1.0320x over previous
"""Trainium2 Bass kernel for nn_CustomSelectAttention (topk_masking).

Computes, for each batch row b:
  read_n  = q[b,n,:] @ w_read[n]   (n = 0,1 only — slots 2-4 unused)
  write_m = k[b,m,:] @ w_write[m]  (m = 0..4)
  s_n[m]  = read_n . write_m / 4 + gumbel_noise_n[b,m]
  out_n   = one_hot(argmax_m s_n[m])
Returns (g1, g2), each [32768, 5] float32 — equal in value to the
reference's straight-through hard gumbel-softmax outputs.

Strategy: pure data parallel over 8 NeuronCores (4096 rows each).
Per core, per 512-row supertile: PE transposes q/k 128x128 chunks (f32,
bit-exact) into PSUM staging, ACT/DVE copies round them to float32r in
SBUF, then weights-stationary float32r matmuls (full PE rate, N=512)
accumulate all 7 projections into one [128, 512] PSUM tile — packed as
4 col-groups of M=32 via block-diagonal weight pairs so output base
partitions stay 32-aligned. A PE transpose-back restores batch-major
layout and the DVE computes scores, adds host-precomputed Gumbel noise
(jax-bit-exact), and emits the one-hot via is_equal against the row max.
"""
import os
import numpy as np

import concourse.bass as bass  # noqa: F401  (engine namespaces live on nc)
import concourse.mybir as mybir
import concourse.tile as tile
from concourse import bacc
from concourse.bass_utils import run_bass_kernel_spmd
from concourse.masks import make_identity

F32 = mybir.dt.float32
F32R = mybir.dt.float32r
P = 128
NCORES = 8
B = 32768
BC = B // NCORES          # rows per core = 4096
NBS = BC // P             # b-subtiles per core = 32
NST = NBS // 4            # supertiles per core = 8 (512 rows each)
D = 512
DK = 16
NW = 5


def gumbel_noise(seed, shape):
    """Bit-exact match of the reference's gumbel noise: computed with the
    same jax ops on the same platform."""
    import jax
    import jax.numpy as jnp
    u = jax.random.uniform(jax.random.key(seed), shape, minval=1e-10, maxval=1.0)
    g = -jnp.log(-jnp.log(u))
    return np.asarray(g, dtype=np.float32)


# Projection t -> (group g, member) packing: pairs (0,1) (2,3) (4,5) share a
# 32-wide col-group via block-diagonal weights; t=6 rides alone in group 3.
def _gm(t):
    return (t // 2, t % 2) if t < 6 else (3, 0)


def _slot(g, cc):
    return g * 8 + cc if g < 3 else 24 + cc


def _build():
    nc = bacc.Bacc(trn_type="TRN2", target_bir_lowering=False, debug=False)
    q_in = nc.dram_tensor("q", [BC, 1024], F32R, kind="ExternalInput").ap()
    k_in = nc.dram_tensor("k", [BC, 2560], F32R, kind="ExternalInput").ap()
    w_in = nc.dram_tensor("w", [P, 896], F32, kind="ExternalInput").ap()
    nz_in = nc.dram_tensor("noise", [P, 2 * NW * NBS], F32, kind="ExternalInput").ap()
    o1 = nc.dram_tensor("out1", [P, NW * NBS], F32, kind="ExternalOutput").ap()
    o2 = nc.dram_tensor("out2", [P, NW * NBS], F32, kind="ExternalOutput").ap()

    qv = q_in.rearrange("(n p) d -> n p d", p=P)   # [32, 128, 1024]
    kv = k_in.rearrange("(n p) d -> n p d", p=P)   # [32, 128, 2560]

    with tile.TileContext(nc) as tc:
        with (
            tc.tile_pool(name="const", bufs=1) as constp,
            tc.tile_pool(name="qin", bufs=8) as qinp,
            tc.tile_pool(name="kin", bufs=11) as kinp,
            tc.tile_pool(name="stage", bufs=4, space="PSUM") as stagep,
            tc.tile_pool(name="qt", bufs=12) as qtp,
            tc.tile_pool(name="pT", bufs=2, space="PSUM") as pTp,
            tc.tile_pool(name="pTs", bufs=2) as pTsp,
            tc.tile_pool(name="pB", bufs=2, space="PSUM") as pBp,
            tc.tile_pool(name="pBs", bufs=2) as pBsp,
            tc.tile_pool(name="ep", bufs=4) as epp,
        ):
            ident_f = constp.tile([P, P], F32)
            make_identity(nc, ident_f)
            ident = constp.tile([P, P], F32R)
            nc.vector.tensor_copy(ident[:], ident_f[:])
            w_sb = constp.tile([P, 896], F32)
            nc.sync.dma_start(w_sb[:], w_in[:])
            w_r = constp.tile([P, 896], F32R)
            nc.vector.tensor_copy(w_r[:], w_sb[:])          # rounds f32 -> f32r
            nz_sb = constp.tile([P, 2 * NW * NBS], F32)
            nc.sync.dma_start(nz_sb[:], nz_in[:])
            out1_sb = constp.tile([P, NW * NBS], F32)
            out2_sb = constp.tile([P, NW * NBS], F32)

            copy_toggle = 0
            for st in range(NST):
                qin = []
                kin = []
                for s in range(4):
                    j = 4 * st + s
                    qt_t = qinp.tile([P, 1024], F32R, tag="qin", name=f"qin_{j}")
                    for sl in range(2):
                        nc.sync.dma_start(qt_t[:, 512 * sl:512 * (sl + 1)],
                                          qv[j][:, 512 * sl:512 * (sl + 1)])
                    qin.append(qt_t)
                for s in range(4):
                    j = 4 * st + s
                    kt_t = kinp.tile([P, 2560], F32R, tag="kin", name=f"kin_{j}")
                    nc.sync.dma_start(kt_t[:, 0:1024], kv[j][:, 0:1024])
                    nc.sync.dma_start(kt_t[:, 1024:2560], kv[j][:, 1024:2560])
                    kin.append(kt_t)

                projT = [None] * 4
                pending = []

                def flush_mm():
                    g_, slot_, cc_, qt_ = pending.pop(0)
                    last_ = (cc_ == 7) if g_ < 3 else (cc_ == 3)
                    nc.tensor.matmul(projT[g_][:],
                                     w_r[:, 32 * slot_: 32 * (slot_ + 1)],
                                     qt_,
                                     start=(cc_ == 0), stop=last_)

                for t in range(7):
                    g, member = _gm(t)
                    for c in range(4):
                        cc = member * 4 + c
                        slot = _slot(g, cc)
                        if projT[g] is None:
                            projT[g] = pTp.tile([32, D], F32, tag="pT", name=f"pT{g}_{st}")
                        stg = stagep.tile([P, D], F32R, tag="stage", name=f"stg_{st}_{t}_{c}")
                        for s in range(4):
                            if t < 2:
                                src = qin[s][:, 512 * t + 128 * c: 512 * t + 128 * (c + 1)]
                            else:
                                src = kin[s][:, 512 * (t - 2) + 128 * c: 512 * (t - 2) + 128 * (c + 1)]
                            nc.tensor.transpose(stg[:, 128 * s:128 * (s + 1)], src, ident[:])
                        qt_r = qtp.tile([P, D], F32R, tag="qt", name=f"qt_{st}_{t}_{c}")
                        if copy_toggle % 2 == 0:
                            nc.vector.tensor_copy(qt_r[:], stg[:])
                        else:
                            nc.scalar.copy(qt_r[:], stg[:])
                        copy_toggle += 1
                        pending.append((g, slot, cc, qt_r[:]))
                        if len(pending) > 2:
                            flush_mm()
                while pending:
                    flush_mm()

                pTs_all = pTsp.tile([P, D], F32, tag="pTs", name=f"pTs_{st}")
                for g in range(4):
                    nc.scalar.copy(pTs_all[32 * g:32 * (g + 1), :], projT[g][:])

                for s in range(4):
                    j = 4 * st + s
                    pB = pBp.tile([P, P], F32, tag="pB", name=f"pB_{j}")
                    nc.tensor.transpose(pB[:], pTs_all[:, 128 * s:128 * (s + 1)], ident_f[:])
                    pBs = pBsp.tile([P, P], F32, tag="pBs", name=f"pBs_{j}")
                    nc.vector.tensor_copy(pBs[:], pB[:])

                    rr = pBs[:, 0:32].rearrange("p (s k) -> p s k", s=2).unsqueeze(2).broadcast_to([P, 2, NW, DK])
                    wv4 = pBs[:, 32:112].rearrange("p (m k) -> p m k", m=NW).unsqueeze(1).broadcast_to([P, 2, NW, DK])
                    prod = epp.tile([P, 2, NW, DK], F32, tag="prod", name=f"prod_{j}")
                    nc.gpsimd.tensor_tensor(prod[:], rr, wv4, op=mybir.AluOpType.mult)
                    sc = epp.tile([P, 2 * NW], F32, tag="sc", name=f"sc_{j}")
                    nc.vector.reduce_sum(sc[:].rearrange("p (s m) -> p s m", s=2), prod[:],
                                         axis=mybir.AxisListType.X)
                    scn = epp.tile([P, 2 * NW], F32, tag="scn", name=f"scn_{j}")
                    nc.gpsimd.tensor_tensor(scn[:], sc[:], nz_sb[:, 2 * NW * j: 2 * NW * (j + 1)],
                                            op=mybir.AluOpType.add)
                    mx = epp.tile([P, 2], F32, tag="mx", name=f"mx_{j}")
                    nc.vector.reduce_max(mx[:], scn[:].rearrange("p (s m) -> p s m", s=2),
                                         axis=mybir.AxisListType.X)
                    nc.vector.tensor_scalar(out1_sb[:, NW * j: NW * (j + 1)], scn[:, 0:NW],
                                            scalar1=mx[:, 0:1], scalar2=None,
                                            op0=mybir.AluOpType.is_equal)
                    nc.vector.tensor_scalar(out2_sb[:, NW * j: NW * (j + 1)], scn[:, NW:2 * NW],
                                            scalar1=mx[:, 1:2], scalar2=None,
                                            op0=mybir.AluOpType.is_equal)

            nc.sync.dma_start(o1[:], out1_sb[:])
            nc.sync.dma_start(o2[:], out2_sb[:])

    nc.compile()
    return nc


_NC_CACHE = []


def _pack_weights(w_read, w_write):
    projw = [np.asarray(w_read[0]) * np.float32(0.25),
             np.asarray(w_read[1]) * np.float32(0.25)] + \
            [np.asarray(w_write[m]) for m in range(NW)]
    W = np.zeros((28, P, 32), np.float32)
    for t in range(7):
        g, member = _gm(t)
        for c in range(4):
            cc = member * 4 + c
            slot = _slot(g, cc)
            W[slot, :, 16 * member:16 * (member + 1)] = projw[t][128 * c:128 * (c + 1), :]
    return np.ascontiguousarray(W.transpose(1, 0, 2).reshape(P, 896))


def kernel(q, k, w_read, w_write):
    q = np.asarray(q, dtype=np.float32)
    k = np.asarray(k, dtype=np.float32)
    w_read = np.asarray(w_read, dtype=np.float32)
    w_write = np.asarray(w_write, dtype=np.float32)

    wt = _pack_weights(w_read, w_write)
    g1 = gumbel_noise(42, (B, NW))
    g2 = gumbel_noise(43, (B, NW))

    in_maps = []
    for ci in range(NCORES):
        rows = slice(BC * ci, BC * (ci + 1))
        qs = np.ascontiguousarray(q[rows, 0:2, :].reshape(BC, 1024))
        ks = np.ascontiguousarray(k[rows].reshape(BC, 2560))
        n1 = g1[rows].reshape(NBS, P, NW).transpose(1, 0, 2)   # [128, 32, 5]
        n2 = g2[rows].reshape(NBS, P, NW).transpose(1, 0, 2)
        nz = np.ascontiguousarray(np.stack([n1, n2], axis=2).reshape(P, 2 * NW * NBS))
        in_maps.append({"q": qs, "k": ks, "w": wt, "noise": nz})

    if not _NC_CACHE:
        _NC_CACHE.append(_build())
    nc = _NC_CACHE[0]

    trace = bool(int(os.environ.get("BASS_KERNEL_TRACE", "0")))
    res = run_bass_kernel_spmd(nc, in_maps, core_ids=list(range(NCORES)), trace=trace)
    kernel.last_exec_time_ns = res.exec_time_ns

    g1_out = np.empty((B, NW), np.float32)
    g2_out = np.empty((B, NW), np.float32)
    for ci in range(NCORES):
        rows = slice(BC * ci, BC * (ci + 1))
        r = res.results[ci]
        g1_out[rows] = r["out1"].reshape(P, NBS, NW).transpose(1, 0, 2).reshape(BC, NW)
        g2_out[rows] = r["out2"].reshape(P, NBS, NW).transpose(1, 0, 2).reshape(BC, NW)
    return (g1_out, g2_out)


kernel.last_exec_time_ns = None
